# revision 2
# baseline (speedup 1.0000x reference)
"""Distributed autoregressive LSTM decoder on 8 TRN2 NeuronCores, v2.

Correctness architecture (fp32r = fp32 rounded to 11 explicit mantissa bits):
- gates: split-precision 3-term matmuls: Whi(f32r)x[hi|lo] two-row lhsT trick +
  Wlo(bf16)x(bf16) accumulated in one PSUM group -> ~2^-21 relative accuracy.
- fc: single coarse f32r pass (error ~3e-7, output tol 5e-3) -> per-bank
  max/max_index directly on PSUM; per-core champion refined via exact fp32
  dot (dyn DMA gather of raw W column + DVE mult/reduce + PE transpose +
  DVE reduce); winner selected on refined values -> reproduces the fp32
  reference argmax trajectory including the 1.6e-7 near-tie at t=104.
- h split on device: u32 add 0x800 + mask (round-half-away 12-bit hi + exact lo).
- emb table pre-transposed/pre-relu'd/pre-split in DRAM: one gather DMA.
- logits written to DRAM directly from PSUM (one strided DMA per step).
"""

from contextlib import ExitStack

import numpy as np

import concourse.bass as bass
import concourse.bacc as bacc
import concourse.mybir as mybir

F32 = mybir.dt.float32
F32R = mybir.dt.float32r
BF16 = mybir.dt.bfloat16
U32 = mybir.dt.uint32

H = 1024
V = 32000
NC = 8
VSH = V // NC        # 4000 vocab rows/core
NB = 8               # fc psum banks
BN = VSH // NB       # 500 cols/bank
GATES = 512
NCH = 8
BIG = 1.0e9
AF = mybir.ActivationFunctionType
OP = mybir.AluOpType


def build_nc(T: int, with_tok_out: bool = False, skip_refine: bool = False, static_gather: bool = False, ladder_sbuf: bool = True):
    nc = bacc.Bacc("TRN2", debug=False)

    d_fcw = nc.dram_tensor("fcw", [128, NCH * VSH], F32R, kind="ExternalInput")
    d_whh_hi = nc.dram_tensor("whh_hi", [128, NCH * GATES], F32R, kind="ExternalInput")
    d_wih_hi = nc.dram_tensor("wih_hi", [128, NCH * GATES], F32R, kind="ExternalInput")
    d_whh_lo = nc.dram_tensor("whh_lo", [128, NCH * GATES], BF16, kind="ExternalInput")
    d_wih_lo = nc.dram_tensor("wih_lo", [128, NCH * GATES], BF16, kind="ExternalInput")
    d_embt = nc.dram_tensor("embt", [128, 16 * V], F32R, kind="ExternalInput")
    d_fcq = nc.dram_tensor("fcq", [128, 8 * VSH], F32, kind="ExternalInput")
    d_h0 = nc.dram_tensor("h0", [128, 16], F32R, kind="ExternalInput")
    d_ident = nc.dram_tensor("ident", [128, 128], F32, kind="ExternalInput")
    d_iotab = nc.dram_tensor("iotab", [1, 8], F32, kind="ExternalInput")
    d_me4k = nc.dram_tensor("me4k", [1, 1], F32, kind="ExternalInput")
    d_out = nc.dram_tensor("out_logits", [T, VSH], F32, kind="ExternalOutput")
    d_tok = (nc.dram_tensor("tok_out", [T, 1], U32, kind="ExternalOutput")
             if with_tok_out else None)

    ctx = ExitStack()
    sb = lambda name, shape, dt=F32: ctx.enter_context(nc.sbuf_tensor(name, shape, dt))
    sem = lambda name: ctx.enter_context(nc.semaphore(name))

    fcw = sb("fcw_sb", [128, NCH * VSH], F32R)
    whh_hi = sb("whh_hi_sb", [128, NCH * GATES], F32R)
    wih_hi = sb("wih_hi_sb", [128, NCH * GATES], F32R)
    whh_lo = sb("whh_lo_sb", [128, NCH * GATES], BF16)
    wih_lo = sb("wih_lo_sb", [128, NCH * GATES], BF16)
    h_all = [sb(f"h_all{p}", [128, 16], F32R) for p in range(2)]
    x_both = [sb(f"x_both{p}", [128, 16], F32R) for p in range(2)]
    x_bf = sb("x_bf", [128, 8], BF16)
    h_bf = sb("h_bf", [128, 8], BF16)
    h_raw = sb("h_raw", [128, 8])
    gates_rd = sb("gates_rd", [128, 4])
    gates_row = sb("gates_row", [1, GATES])
    logits_sb = sb("logits_sb", [1, VSH])
    sig_ifo = sb("sig_ifo", [128, 3])
    tanh_g = sb("tanh_g", [128, 1])
    tanh_c = sb("tanh_c", [128, 1])
    tmp1 = sb("tmp1", [128, 1])
    tmp2 = sb("tmp2", [128, 1])
    c_state = sb("c_state", [128, 1])
    h_new = sb("h_new", [128, 1])
    h_send = sb("h_send", [128, 2])
    refine_w = sb("refine_w", [128, 8])
    prods = sb("prods", [128, 8])
    partial = sb("partial", [128, 1])
    bval8 = sb("bval8", [1, 64])
    bidx8 = sb("bidx8", [1, 64], U32)
    bidxf = sb("bidxf", [1, 8])
    qrow = sb("qrow", [1, 8])
    vmax = sb("vmax", [1, 1])
    eqw = sb("eqw", [1, 8], U32)
    maskq = sb("maskq", [1, 8])
    qstar = sb("qstar", [1, 1])
    qstar_u = sb("qstar_u", [1, 1], U32)
    cand_send = sb("cand_send", [128, 2])
    cands = [sb(f"cands{p}", [128, 16]) for p in range(2)]
    wmax = sb("wmax", [1, 1])
    eqw2 = sb("eqw2", [1, 8], U32)
    maskw = sb("maskw", [1, 8])
    tokf = sb("tokf", [1, 1])
    tok_u = sb("tok_u", [1, 1], U32)
    ident = sb("ident_sb", [128, 128])
    iotab = sb("iotab_sb", [1, 8])
    me4k = sb("me4k_sb", [1, 1])
    bigc = sb("bigc", [1, 8])

    ps = ctx.enter_context(nc.psum_tensor("ps", [128, 4096], F32))
    # fc bank b: ps[0:1, 512b:512b+500] (row 0 only); gates: ps[32:34, 512:1024]
    # rows 2-3 (bank 1, partition-disjoint from fc); transpose: ps[32:33, 0:128].

    s_pre = sem("s_pre")
    s_h = [sem(f"s_h{i}") for i in range(NC)]
    s_cand = [sem(f"s_cand{i}") for i in range(NC)]
    s_lsendh = sem("s_lsendh")
    s_lsendc = sem("s_lsendc")
    s_hready = sem("s_hready")
    s_cready = sem("s_cready")   # also: refined-read done (PE transpose guard)
    s_tokw = sem("s_tokw")
    s_gath = sem("s_gath")       # emb gather DMA (16/step, preload +16)
    s_xbf = sem("s_xbf")
    s_hbf = sem("s_hbf")
    s_pe_g = sem("s_pe_g")       # gates group complete (1/step)
    s_pe_fc = sem("s_pe_fc")     # fc bank stops (8/step)
    s_pe_t = sem("s_pe_t")       # transpose done (1/step)
    s_gred = sem("s_gred")       # gates redistribute DMA (16/step)
    s_gdr = sem("s_gdr")
    s_adrn = sem("s_adrn")
    s_actp = sem("s_actp")       # sigmoid+tanh (2/step)
    s_dvec = sem("s_dvec")       # c_state (1/step)
    s_actc = sem("s_actc")       # tanh_c (1/step)
    s_hnew = sem("s_hnew")
    s_lad = sem("s_lad")         # bank ladder ops (8/step, on max_index)
    s_qstar = sem("s_qstar")
    s_refw = sem("s_refw")       # refine gather DMA (16/step)
    s_partial = sem("s_partial")
    s_out = sem("s_out")         # output row DMA (16/step)
    s_tokd = sem("s_tokd")
    s_gpre = sem("s_gpre")
    s_prep = sem("s_prep")

    PRE_TOTAL = 16 * 9 + 3       # 9 hwdge preload DMAs + 3 DVE memsets

    bank_order = [1, 0, 2, 3, 4, 5, 6, 7]

    with nc.Block() as block:

        # ================= SYNC =================
        @block.sync
        def _(g: bass.BassEngine):
            g.dma_start(fcw[:, 0 : NCH * VSH // 2], d_fcw[:, 0 : NCH * VSH // 2]).then_inc(s_pre, 16)
            g.dma_start(h_all[0][:, :], d_h0[:, :]).then_inc(s_pre, 16)
            g.dma_start(x_both[0][:, :], d_embt[:, 0:16]).then_inc(s_pre, 16)
            g.dma_start(ident[:, :], d_ident[:, :]).then_inc(s_pre, 16)
            g.wait_ge(s_pre, PRE_TOTAL)
            embt_v = d_embt.rearrange("p (v k) -> p v k", k=16)
            fcq_v = d_fcq.rearrange("p (q c) -> p q c", c=8)
            for t in range(T):
                pj = (t + 1) % 2
                # gates redistribute: row -> [128,4]
                g.wait_ge(s_gdr, t + 1)
                g.dma_start(
                    gates_rd[:, 0:4],
                    gates_row[0:1, :].rearrange("a (p gt) -> a p gt", p=128),
                ).then_inc(s_gred, 16)
                # logits row -> DRAM
                g.wait_ge(s_adrn, NB * (t + 1))
                g.dma_start(d_out[t : t + 1, :], logits_sb[0:1, :]).then_inc(s_out, 16)
                if t < T - 1:
                    if not skip_refine:
                        # refine gather: fcw[:, q::4000] -> [128,8]
                        g.wait_ge(s_qstar, t + 1)
                        qv = g.value_load(qstar_u[0:1, 0:1])
                        g.dma_start(
                            refine_w[:, :], fcq_v[:, bass.ds(qv, 1), :]
                        ).then_inc(s_refw, 16)
                    # emb gather
                    g.wait_ge(s_tokw, t + 1)
                    if static_gather:
                        g.dma_start(
                            x_both[pj][:, :], d_embt[:, 0:16]
                        ).then_inc(s_gath, 16)
                    else:
                        tok = g.value_load(tok_u[0:1, 0:1])
                        g.dma_start(
                            x_both[pj][:, :], embt_v[:, bass.ds(tok, 1), :]
                        ).then_inc(s_gath, 16)
                    if with_tok_out:
                        g.dma_start(d_tok[t : t + 1, :], tok_u[0:1, 0:1]).then_inc(s_tokd, 16)
            g.wait_ge(s_out, 16 * T)
            g.wait_ge(s_gred, 16 * T)
            if with_tok_out and T > 1:
                g.wait_ge(s_tokd, 16 * (T - 1))

        # ================= SCALAR (ACT) =================
        @block.scalar
        def _(g: bass.BassScalarEngine):
            g.dma_start(whh_hi[:, :], d_whh_hi[:, :]).then_inc(s_pre, 16)
            g.dma_start(wih_hi[:, :], d_wih_hi[:, :]).then_inc(s_pre, 16)
            g.dma_start(whh_lo[:, :], d_whh_lo[:, :]).then_inc(s_pre, 16)
            g.dma_start(wih_lo[:, :], d_wih_lo[:, :]).then_inc(s_pre, 16)
            g.dma_start(fcw[:, NCH * VSH // 2 :], d_fcw[:, NCH * VSH // 2 :]).then_inc(s_pre, 16)
            g.wait_ge(s_pre, PRE_TOTAL)
            # initial converts from preloaded h0/x0
            g.activation(h_bf[:, :], h_all[0][:, 0:16:2].bitcast(F32), AF.Copy).then_inc(s_hbf, 1)
            g.activation(x_bf[:, :], x_both[0][:, 0:8].bitcast(F32), AF.Copy).then_inc(s_xbf, 1)
            for t in range(T):
                pi = t % 2
                pj = (t + 1) % 2
                # gates psum row -> SBUF
                g.wait_ge(s_pe_g, t + 1)
                g.activation(gates_row[0:1, :], ps[0:1, 512:1024], AF.Copy).then_inc(s_gdr, 1)
                # pointwise activations for step t
                g.wait_ge(s_gred, 16 * (t + 1))
                g.activation(sig_ifo[:, :], gates_rd[:, 0:3], AF.Sigmoid)
                g.activation(tanh_g[:, :], gates_rd[:, 3:4], AF.Tanh).then_inc(s_actp, 1)
                g.wait_ge(s_dvec, t + 1)
                g.activation(tanh_c[:, :], c_state[:, :], AF.Tanh).then_inc(s_actc, 1)
                # h_bf for h_{t+1} after its broadcast
                for c in range(NC):
                    g.wait_ge(s_h[c], 2 * (t + 1))
                g.activation(h_bf[:, :], h_all[pj][:, 0:16:2].bitcast(F32), AF.Copy).then_inc(s_hbf, 1)
                # fc bank drains
                for k, b in enumerate(bank_order):
                    g.wait_ge(s_pe_fc, NB * t + k + 1)
                    g.activation(
                        logits_sb[0:1, BN * b : BN * (b + 1)],
                        ps[0:1, 512 * b : 512 * b + BN], AF.Copy,
                    ).then_inc(s_adrn, 1)
                # x_bf for x_{t+1} after gather
                if t < T - 1:
                    g.wait_ge(s_gath, 16 * (t + 1))
                    g.activation(x_bf[:, :], x_both[pj][:, 0:8].bitcast(F32), AF.Copy).then_inc(s_xbf, 1)

        # ================= VECTOR (DVE) =================
        @block.vector
        def _(g: bass.BassVectorEngine):
            g.memset(c_state[:, :], 0.0).then_inc(s_pre, 1)
            g.memset(bigc[:, :], BIG).then_inc(s_pre, 1)
            g.memset(cand_send[:, :], 0.0).then_inc(s_pre, 1)
            g.wait_ge(s_pre, PRE_TOTAL)
            g.wait_ge(s_gpre, 32)
            for t in range(T):
                pi = t % 2
                pj = (t + 1) % 2
                g.wait_ge(s_actp, t + 1)
                g.tensor_tensor(tmp1[:, :], sig_ifo[:, 1:2], c_state[:, :], OP.mult)
                g.tensor_tensor(tmp2[:, :], sig_ifo[:, 0:1], tanh_g[:, :], OP.mult)
                g.drain()
                g.tensor_tensor(c_state[:, :], tmp1[:, :], tmp2[:, :], OP.add).then_inc(s_dvec, 1)
                g.wait_ge(s_actc, t + 1)
                g.tensor_tensor(h_new[:, :], sig_ifo[:, 2:3], tanh_c[:, :], OP.mult).then_inc(s_hnew, 1)
                g.drain()
                # ---- h split (Veltkamp, 12-bit hi): p=h*(2^12+1); hi=p-(p-h)
                if t > 0:
                    g.wait_ge(s_lsendh, 16 * t)
                g.tensor_scalar(
                    tmp1[:, :], h_new[:, :], 4097.0, scalar2=None, op0=OP.mult,
                )
                g.drain()
                g.tensor_tensor(tmp2[:, :], tmp1[:, :], h_new[:, :], OP.subtract)
                g.drain()
                g.tensor_tensor(h_send[:, 0:1], tmp1[:, :], tmp2[:, :], OP.subtract)
                g.drain()
                g.tensor_tensor(h_send[:, 1:2], h_new[:, :], h_send[:, 0:1], OP.subtract).then_inc(s_hready, 1)
                # ---- h_raw for refine (h_{t+1} after broadcast arrives)
                for c in range(NC):
                    g.wait_ge(s_h[c], 2 * (t + 1))
                g.tensor_tensor(
                    h_raw[:, :], h_all[pj][:, 0:16:2].bitcast(F32),
                    h_all[pj][:, 1:16:2].bitcast(F32), OP.add,
                )
                g.drain()
                if t < T - 1:
                    # ---- per-bank coarse ladder
                    for k, b in enumerate(bank_order):
                        if ladder_sbuf:
                            g.wait_ge(s_adrn, NB * t + k + 1)
                            lsrc = logits_sb[0:1, BN * b : BN * (b + 1)]
                        else:
                            g.wait_ge(s_pe_fc, NB * t + k + 1)
                            lsrc = ps[0:1, 512 * b : 512 * b + BN]
                        g.max(bval8[0:1, 8 * b : 8 * b + 8], lsrc)
                        g.drain()
                        g.max_index(
                            bidx8[0:1, 8 * b : 8 * b + 8], bval8[0:1, 8 * b : 8 * b + 8],
                            lsrc,
                        ).then_inc(s_lad, 1)
                    g.drain()
                    # ---- cross-bank champion: q* and global id
                    g.tensor_reduce(vmax[0:1, :], bval8[0:1, 0:64:8], mybir.AxisListType.X, OP.max)
                    g.tensor_copy(bidxf[0:1, :], bidx8[0:1, 0:64:8])
                    g.drain()
                    g.tensor_tensor(qrow[0:1, :], bidxf[0:1, :], iotab[0:1, :], OP.add)
                    g.tensor_scalar(
                        eqw[0:1, :], bval8[0:1, 0:64:8], vmax[0:1, 0:1],
                        scalar2=None, op0=OP.is_equal,
                    )
                    g.drain()
                    g.select(maskq[0:1, :], eqw[0:1, :], qrow[0:1, :], bigc[0:1, :], add_drain=True)
                    g.drain()
                    g.tensor_reduce(qstar[0:1, :], maskq[0:1, :], mybir.AxisListType.X, OP.min)
                    g.drain()
                    g.tensor_copy(qstar_u[0:1, :], qstar[0:1, :]).then_inc(s_qstar, 1)
                    if t > 0:
                        g.wait_ge(s_lsendc, 16 * t)
                    g.tensor_tensor(cand_send[0:1, 1:2], qstar[0:1, :], me4k[0:1, :], OP.add)
                    if skip_refine:
                        g.tensor_copy(cand_send[0:1, 0:1], vmax[0:1, :]).then_inc(s_cready, 1)
                    else:
                        # ---- refine: exact dot of raw W col q* with h_raw
                        g.wait_ge(s_refw, 16 * (t + 1))
                        g.tensor_tensor(prods[:, :], refine_w[:, :], h_raw[:, :], OP.mult)
                        g.drain()
                        g.tensor_reduce(partial[:, :], prods[:, :], mybir.AxisListType.X, OP.add).then_inc(s_partial, 1)
                        g.wait_ge(s_pe_t, t + 1)
                        g.tensor_reduce(
                            cand_send[0:1, 0:1], ps[0:1, 0:128], mybir.AxisListType.X, OP.add
                        ).then_inc(s_cready, 1)
                    # ---- winner over refined champions
                    for i in range(NC):
                        g.wait_ge(s_cand[i], 2 * (t + 1))
                    cp = cands[t % 2]
                    g.tensor_reduce(wmax[0:1, :], cp[0:1, 0:16:2], mybir.AxisListType.X, OP.max)
                    g.drain()
                    g.tensor_scalar(
                        eqw2[0:1, :], cp[0:1, 0:16:2], wmax[0:1, 0:1],
                        scalar2=None, op0=OP.is_equal,
                    )
                    g.drain()
                    g.select(maskw[0:1, :], eqw2[0:1, :], cp[0:1, 1:16:2], bigc[0:1, :], add_drain=True)
                    g.drain()
                    g.tensor_reduce(tokf[0:1, :], maskw[0:1, :], mybir.AxisListType.X, OP.min)
                    g.drain()
                    g.tensor_copy(tok_u[0:1, :], tokf[0:1, :]).then_inc(s_tokw, 1)
            g.wait_ge(s_lsendh, 16 * T)
            if T > 1:
                g.wait_ge(s_lsendc, 16 * (T - 1))

        # ================= TENSOR (PE) =================
        @block.tensor
        def _(g: bass.BassTensorEngine):
            g.wait_ge(s_pre, PRE_TOTAL)
            g.wait_ge(s_hbf, 1)
            # gates-h for t=0 (group start)
            for half in (0, 1):
                for c in range(NCH):
                    g.matmul(
                        ps[0:1, 512:1024], h_all[0][:, 2 * c + half : 2 * c + half + 1],
                        whh_hi[:, GATES * c : GATES * (c + 1)],
                        start=(half == 0 and c == 0), stop=False, skip_group_check=True,
                    )
            for c in range(NCH):
                g.matmul(
                    ps[0:1, 512:1024], h_bf[:, c : c + 1],
                    whh_lo[:, GATES * c : GATES * (c + 1)],
                    start=False, stop=False, skip_group_check=True,
                )
            for t in range(T):
                pi = t % 2
                pj = (t + 1) % 2
                # ---- gates-x for step t (group end)
                if t > 0:
                    g.wait_ge(s_gath, 16 * t)
                g.wait_ge(s_xbf, t + 1)
                for half in (0, 1):
                    for c in range(NCH):
                        g.matmul(
                            ps[0:1, 512:1024], x_both[pi][:, 8 * half + c : 8 * half + c + 1],
                            wih_hi[:, GATES * c : GATES * (c + 1)],
                            start=False, stop=False, skip_group_check=True,
                        )
                for c in range(NCH):
                    mm = g.matmul(
                        ps[0:1, 512:1024], x_bf[:, c : c + 1],
                        wih_lo[:, GATES * c : GATES * (c + 1)],
                        start=False, stop=(c == NCH - 1), skip_group_check=True,
                    )
                    if c == NCH - 1:
                        mm.then_inc(s_pe_g, 1)
                # ---- fc on h_{t+1}
                for c in range(NC):
                    g.wait_ge(s_h[c], 2 * (t + 1))
                if t > 0:
                    g.wait_ge(s_adrn, NB * t)  # prev bank drains done
                    g.wait_ge(s_lad, 8 * t)    # prev ladders done
                for k, b in enumerate(bank_order):
                    for c in range(NCH):
                        mm = g.matmul(
                            ps[0:1, 512 * b : 512 * b + BN],
                            h_all[pj][:, 2 * c : 2 * c + 1],
                            fcw[:, VSH * c + BN * b : VSH * c + BN * (b + 1)],
                            start=(c == 0), stop=(c == NCH - 1),
                            skip_group_check=True,
                        )
                        if c == NCH - 1:
                            mm.then_inc(s_pe_fc, 1)
                # ---- gates-h for t+1 (group start)
                if t < T - 1:
                    g.wait_ge(s_hbf, t + 2)
                    g.wait_ge(s_gdr, t + 1)    # prev gates row drained
                    g.wait_ge(s_adrn, NB * t + 1)  # bank1 drained (first in order)
                    g.wait_ge(s_lad, 8 * t + 1)    # bank1 ladder done
                    for half in (0, 1):
                        for c in range(NCH):
                            g.matmul(
                                ps[0:1, 512:1024], h_all[pj][:, 2 * c + half : 2 * c + half + 1],
                                whh_hi[:, GATES * c : GATES * (c + 1)],
                                start=(half == 0 and c == 0), stop=False, skip_group_check=True,
                            )
                    for c in range(NCH):
                        g.matmul(
                            ps[0:1, 512:1024], h_bf[:, c : c + 1],
                            whh_lo[:, GATES * c : GATES * (c + 1)],
                            start=False, stop=False, skip_group_check=True,
                        )
                    if not skip_refine:
                        # ---- transpose refine partial -> bank0 row 0 (after bank0
                        # ladder + drain consumed; next fc start is token-gated)
                        g.wait_ge(s_partial, t + 1)
                        g.wait_ge(s_lad, 8 * t + 2)
                        g.wait_ge(s_adrn, NB * t + 2)
                        g.transpose(ps[0:1, 0:128], partial[:, 0:1], ident[:, :]).then_inc(s_pe_t, 1)

        # ================= GPSIMD =================
        @block.gpsimd
        def _(g: bass.BassGpSimd):
            from concourse import library_config
            g.load_library(library_config.remote_dma)
            g.dma_start(iotab[:, :], d_iotab[:, :]).then_inc(s_gpre, 16)
            g.dma_start(me4k[:, :], d_me4k[:, :]).then_inc(s_gpre, 16)
            g.wait_ge(s_pre, PRE_TOTAL)
            g.wait_ge(s_gpre, 32)
            pid = g.partition_id()
            for case in g.Switch(pid, NC):
                prep = 0
                for t in range(T):
                    pi = t % 2
                    pj = (t + 1) % 2
                    g.remote_dma_broadcast(
                        out_ap=h_all[pj][:, 2 * case : 2 * case + 2],
                        in_ap=h_send[:, 0:2].bitcast(F32R),
                        remote_sem=s_h[case],
                        local_sem=s_lsendh,
                        rdests=[(0, k) for k in range(NC)],
                    ).then_inc(s_prep, 1)
                    prep += 1
                    g.wait_ge(s_prep, prep)
                    g.wait_ge(s_hready, t + 1)
                    g.trigger_dma(1)
                    if t < T - 1:
                        g.remote_dma_broadcast(
                            out_ap=cands[t % 2][:, 2 * case : 2 * case + 2],
                            in_ap=cand_send[:, 0:2],
                            remote_sem=s_cand[case],
                            local_sem=s_lsendc,
                            rdests=[(0, k) for k in range(NC)],
                        ).then_inc(s_prep, 1)
                        prep += 1
                        g.wait_ge(s_prep, prep)
                        g.wait_ge(s_cready, t + 1)
                        g.trigger_dma(1)
                g.wait_ge(s_lsendh, 16 * T)
                if T > 1:
                    g.wait_ge(s_lsendc, 16 * (T - 1))

    nc.has_collectives = True
    ctx.close()
    nc.compile()
    return nc


# ======================= host-side prep =======================

def _fp32r_rne(x):
    b = np.ascontiguousarray(np.asarray(x, np.float32)).view(np.uint32)
    low = b & np.uint32(0xFFF)
    keep = b & np.uint32(0xFFFFF000)
    lsb = (b >> np.uint32(12)) & np.uint32(1)
    up = (low > 0x800) | ((low == 0x800) & (lsb == 1))
    return (keep + (up.astype(np.uint32) << np.uint32(12))).view(np.float32)


def _rha12(x):
    b = np.ascontiguousarray(np.asarray(x, np.float32)).view(np.uint32)
    return ((b + np.uint32(0x800)) & np.uint32(0xFFFFF000)).view(np.float32)


def prep_core_inputs(inp: dict, T: int):
    import ml_dtypes

    fc_W = np.asarray(inp["fc_W"], np.float32)
    W_ih = np.asarray(inp["W_ih"], np.float32)
    W_hh = np.asarray(inp["W_hh"], np.float32)
    emb = np.asarray(inp["emb"], np.float32)
    b_ih = np.asarray(inp["b_ih"], np.float32)
    b_hh = np.asarray(inp["b_hh"], np.float32)
    fc_b = np.asarray(inp["fc_b"], np.float32)
    assert not np.any(b_ih) and not np.any(b_hh) and not np.any(fc_b), \
        "bias support not implemented in v2"
    l1_W = np.asarray(inp["l1_W"], np.float64)
    l1_b = np.asarray(inp["l1_b"], np.float64)
    z = np.asarray(inp["z"], np.float64)
    c_ = np.asarray(inp["c"], np.float64)

    h0 = (l1_W @ np.concatenate([z, c_]) + l1_b).astype(np.float32)  # [1024]
    h0_hi = _rha12(h0)
    h0_lo = (h0.astype(np.float64) - h0_hi.astype(np.float64)).astype(np.float32)
    h0_both = np.zeros((128, 16), np.float32)
    for c in range(8):
        h0_both[:, 2 * c] = h0_hi[128 * c : 128 * (c + 1)]
        h0_both[:, 2 * c + 1] = h0_lo[128 * c : 128 * (c + 1)]

    # emb table: pre-relu, pre-split, pre-transposed: embt[p, 16v+c]=hi, +8+c=lo
    R = np.maximum(emb, 0)
    Rhi = _rha12(R)
    Rlo = (R.astype(np.float64) - Rhi.astype(np.float64)).astype(np.float32)
    embt = np.empty((128, V, 16), np.float32)
    embt[:, :, 0:8] = Rhi.reshape(V, 8, 128).transpose(2, 0, 1)
    embt[:, :, 8:16] = Rlo.reshape(V, 8, 128).transpose(2, 0, 1)
    embt = np.ascontiguousarray(embt.reshape(128, 16 * V))

    ident = np.eye(128, dtype=np.float32)
    iotab = (np.arange(8, dtype=np.float32) * BN).reshape(1, 8)

    go = np.array([0, 1, 3, 2])  # gate-block order i,f,o,g -> pytorch rows i,f,g,o
    n = np.arange(GATES)

    def split_w(Wsel):
        hi = _fp32r_rne(Wsel)
        lo = (Wsel.astype(np.float64) - hi.astype(np.float64)).astype(np.float32)
        return hi, lo.astype(ml_dtypes.bfloat16)

    maps = []
    for me in range(NC):
        fcw = np.empty((128, NCH * VSH), np.float32)
        fcsel = fc_W[me * VSH : (me + 1) * VSH]          # [4000, 1024]
        for c in range(NCH):
            fcw[:, c * VSH : (c + 1) * VSH] = fcsel[:, 128 * c : 128 * (c + 1)].T
        fcq = np.ascontiguousarray(
            fcsel.reshape(VSH, 8, 128).transpose(2, 0, 1).reshape(128, 8 * VSH))
        grow = go[n % 4] * H + 128 * me + (n // 4)
        whsel = W_hh[grow]                                # [512, 1024]
        wisel = W_ih[grow]
        wh_hi, wh_lo = split_w(whsel)
        wi_hi, wi_lo = split_w(wisel)
        whh_hi = np.empty((128, NCH * GATES), np.float32)
        wih_hi = np.empty((128, NCH * GATES), np.float32)
        import ml_dtypes as md
        whh_lo = np.empty((128, NCH * GATES), md.bfloat16)
        wih_lo = np.empty((128, NCH * GATES), md.bfloat16)
        for c in range(NCH):
            sl = slice(c * GATES, (c + 1) * GATES)
            whh_hi[:, sl] = wh_hi[:, 128 * c : 128 * (c + 1)].T
            wih_hi[:, sl] = wi_hi[:, 128 * c : 128 * (c + 1)].T
            whh_lo[:, sl] = wh_lo[:, 128 * c : 128 * (c + 1)].T
            wih_lo[:, sl] = wi_lo[:, 128 * c : 128 * (c + 1)].T
        maps.append(
            dict(
                fcw=np.ascontiguousarray(fcw), fcq=fcq,
                whh_hi=np.ascontiguousarray(whh_hi),
                wih_hi=np.ascontiguousarray(wih_hi),
                whh_lo=np.ascontiguousarray(whh_lo),
                wih_lo=np.ascontiguousarray(wih_lo),
                embt=embt, h0=h0_both, ident=ident, iotab=iotab,
                me4k=np.full((1, 1), me * VSH, np.float32),
            )
        )
    return maps


def assemble_output(results, T: int):
    out = np.empty((T, 1, V), np.float32)
    for me, r in enumerate(results):
        out[:, 0, me * VSH : (me + 1) * VSH] = r["out_logits"]
    return out


# ======================= public entry point =======================

def kernel(**inputs):
    from concourse.bass_utils import run_bass_kernel_spmd

    T = int(inputs.get("max_length", 128))
    assert T == 128, f"kernel compiled for max_length=128, got {T}"
    inp = {k: (np.asarray(v) if hasattr(v, "shape") or not np.isscalar(v) else v)
           for k, v in inputs.items()}
    maps = prep_core_inputs(inp, T)
    nc = build_nc(T)
    res = run_bass_kernel_spmd(nc, maps, core_ids=list(range(NC)))
    return assemble_output(res.results, T)
